# revision 42
# baseline (speedup 1.0000x reference)
"""Based linear-attention via chunked state form on 8 TRN2 NeuronCores.

Sharding: core c handles batch b = c // 4 and head-group g = c % 4
(3 of 12 heads).  Wq/Wk/Wv column-split by head, Wo row-split; each
core emits a partial [L, D] output and the host sums the 4 partials
per batch.

Algorithm: attn = 1 + s + 0.5 s^2 (s = q.k/sqrt(F)) is an exact
feature-map kernel phi(q).phi(k) with phi = [1, x, vec(x (x) x)]
(dim 1+16+256).  Chunked linear attention with C=128: per chunk the
intra part is one masked 128x128 quadratic block; the cross-chunk
part contracts phi(q) against a running state S = sum phi(k) (x)
[v | 1] per head ([2*17 + 256 rows, 129 cols]; column 128 carries
the causal normalizer z).  Everything is scaled 2x (attn2 = 2 + 2s
+ s^2 = (0.5*s2+1)^2 + 1 with s2 = 2s via Wq scaled by 0.5) so the
intra path stays one Square activation; the 2x cancels in o/z.

Matmul operands must sit at partition base 0/32/64 with equal bases
for lhsT/rhs, so qT/kT live in separate [96, L] tiles with head h in
rows 32h..32h+15; row 32h+16 holds the constant feature (2.0 in qT,
1.0 in kT), which merges the "1" feature into the 17-row ab state
block.  phi(q)'s 256 outer-product rows are built PE-side with 0/1
replication selectors + one DVE multiply (no psum->sbuf copy);
phi(k) gets l-major layout from one DMA transpose per chunk.
"""

import sys

sys.path.insert(0, "/opt/trn_rl_repo")

from contextlib import ExitStack

import ml_dtypes
import numpy as np

import concourse.bass as bass
import concourse.tile as tile
from concourse import bacc, mybir
from concourse.bass_utils import run_bass_kernel_spmd
from concourse.masks import make_identity

B, L, D = 2, 2048, 1536
H, FDIM, HD = 12, 16, 128
NH = 3            # heads per core
P = 128
NK = D // P       # 12 contraction tiles
NCH = L // P      # 16 chunks
GQ = 96           # padded q rows (3 heads x 32); same for k
DV = NH * HD      # 384 v cols per core
DVA = HD + 1      # 129: v columns + ones column per head

DT = mybir.dt.bfloat16
NPDT = ml_dtypes.bfloat16
F32 = mybir.dt.float32

_ADD = mybir.AluOpType.add
_MULT = mybir.AluOpType.mult
_SQUARE = mybir.ActivationFunctionType.Square
_COPY = mybir.ActivationFunctionType.Copy


def _build():
    nc = bacc.Bacc("TRN2", target_bir_lowering=False, debug=False, num_devices=8)

    hsT = nc.dram_tensor("hsT", [D, L], DT, kind="ExternalInput").ap()
    wqk = nc.dram_tensor("wqk", [D, GQ], DT, kind="ExternalInput").ap()
    wv = nc.dram_tensor("wv", [D, DV], DT, kind="ExternalInput").ap()
    wo = nc.dram_tensor("wo", [DV, D], DT, kind="ExternalInput").ap()
    maskd = nc.dram_tensor("maskd", [P, P], DT, kind="ExternalInput").ap()
    rseld = nc.dram_tensor("rseld", [GQ, 3 * P + 2], DT, kind="ExternalInput").ap()
    out = nc.dram_tensor("out", [L, D], DT, kind="ExternalOutput").ap()

    with tile.TileContext(nc, trace_sim=False) as tc, ExitStack() as ctx:
        cpool = ctx.enter_context(tc.tile_pool(name="consts", bufs=1))
        wqk_sb = cpool.tile([P, NK * GQ], DT, tag="wqk")
        wv_sb = cpool.tile([P, NK * DV], DT, tag="wv")
        wo_sb = cpool.tile([P, NH * D], DT, tag="wo")
        mask_sb = cpool.tile([P, P], DT, tag="mask")
        rsel_sb = cpool.tile([GQ, 3 * P + 2], DT, tag="rsel")
        ident = cpool.tile([P, P], F32, tag="ident")
        identb = cpool.tile([P, P], DT, tag="identb")
        hs_all = cpool.tile([P, NK * L], DT, tag="hs")     # [p, (k l)]
        qT_sb = cpool.tile([GQ, L], DT, tag="qT")
        pk_sb = cpool.tile([GQ, L], DT, tag="pk")
        kT_sb = cpool.tile([GQ, L], DT, tag="kT")
        v_sb = cpool.tile([P, NCH * NH * DVA], DT, tag="v")  # [l, (c h dv)]
        S_sb = [cpool.tile([P, NH * NH * DVA], DT, tag=f"S{u}", name=f"S{u}")
                for u in range(2)]   # head h at cols h*387..(h+1)*387

        # ---- input DMA (few big ops; strip 0 first so chunk 0 can start)
        nc.sync.dma_start(wqk_sb[:].rearrange("p (k g) -> p k g", k=NK),
                          wqk.rearrange("(k p) g -> p k g", p=P))
        SW = 4 * P
        hs3 = hs_all[:].rearrange("p (k l) -> p k l", k=NK)
        hsT3 = hsT.rearrange("(k p) l -> p k l", p=P)
        for s in range(4):
            nc.sync.dma_start(hs3[:, :, s * SW:(s + 1) * SW],
                              hsT3[:, :, s * SW:(s + 1) * SW])
            if s == 0:
                nc.sync.dma_start(rsel_sb[:], rseld)
                nc.sync.dma_start(mask_sb[:], maskd)
                nc.sync.dma_start(wv_sb[:].rearrange("p (k g) -> p k g", k=NK),
                                  wv.rearrange("(k p) g -> p k g", p=P))
            elif s == 1:
                nc.sync.dma_start(wo_sb[:].rearrange("p (h d) -> p h d", h=NH),
                                  wo.rearrange("(h p) d -> p h d", p=P))
        make_identity(nc, ident[:])
        make_identity(nc, identb[:])
        # constant feature rows: 2.0 in qT (the "2" of attn2), 1.0 in kT.
        # Whole 32-row groups (partition alignment); the real q/k rows are
        # overwritten by the projection copies before any read.
        for h in range(NH):
            nc.vector.memset(qT_sb[32 * h: 32 * h + 32, :], 2.0)
            nc.vector.memset(kT_sb[32 * h: 32 * h + 32, :], 1.0)
        # ones column of v_aug, all chunks/heads at once
        nc.vector.memset(
            v_sb[:].rearrange("p (c h x) -> p c h x", c=NCH, h=NH)[:, :, :, HD], 1.0)
        for u in range(2):
            nc.vector.memset(S_sb[u][:], 0.0)

        # ---- PSUM: exactly 8 banks
        ppool = ctx.enter_context(tc.tile_pool(name="ps", bufs=1, space="PSUM"))
        pb1 = ppool.tile([P, 512], F32, tag="pb1")   # qk + slots A,B,E1
        pb2 = ppool.tile([P, 512], F32, tag="pb2")   # v + slots C,C2,E2
        S_ps = [ppool.tile([P, NH * DVA], F32, tag=f"Sp{h}", name=f"Sp{h}")
                for h in range(NH)]                   # 3 banks
        o_ps = ppool.tile([P, NH * DVA], F32, tag="o_ps")  # 1 bank
        op_ps = [ppool.tile([P, 512], F32, tag=f"op{i}", name=f"op{i}")
                 for i in range(2)]                   # oproj double buffer

        qk_ps = op_ps[0][0:GQ, 0:512]   # time-shared with oproj buf 0
        slotA = pb1[:, 0:128]
        slotB = pb1[:, 128:256]
        slotC = pb1[:, 256:384]
        slotD = pb1[:, 384:512]
        v_ps = pb2[:, 0:DV]
        slotE = pb2[:, DV:DV + 128]
        sT_slots = [slotE, slotA, slotB]
        q2_slots = [[slotA, slotB, slotC], [slotE, slotC, slotD],
                    [slotE, slotA, slotB]]
        q2_pairs = [pb1[:, 128:384], pb1[:, 256:512], pb1[:, 0:256]]
        oT_slots = [slotC.bitcast(DT)[:, 0:P], slotD.bitcast(DT)[:, 0:P],
                    slotE.bitcast(DT)[:, 0:P]]

        # dead regions of the S psum ab-block must read as 0
        for h in range(NH):
            nc.vector.memset(S_ps[h][:, 0:DVA], 0.0)

        wpool = ctx.enter_context(tc.tile_pool(name="work", bufs=1))
        attT = [[wpool.tile([P, P], DT, tag=f"attT{u}{h}", name=f"attT{u}{h}")
                 for h in range(NH)] for u in range(2)]
        fq2T = [[wpool.tile([P, 2 * P], DT, tag=f"fq2T{u}{h}", name=f"fq2T{u}{h}")
                 for h in range(NH)] for u in range(2)]
        fk2 = [wpool.tile([P, NH * 2 * P], DT, tag=f"fk2{u}", name=f"fk2{u}")
               for u in range(2)]    # head h at cols h*256..(h+1)*256
        k_rm = [wpool.tile([P, GQ], DT, tag=f"krm{u}", name=f"krm{u}")
                for u in range(2)]
        rep_sb = [[wpool.tile([P, P], DT, tag=f"rep{u}{h}", name=f"rep{u}{h}")
                   for h in range(NH)] for u in range(2)]
        o_sb = [[wpool.tile([P, P], DT, tag=f"o{u}{h}", name=f"o{u}{h}")
                 for h in range(NH)] for u in range(2)]
        oT_sb = [[wpool.tile([P, P], DT, tag=f"oT{u}{h}", name=f"oT{u}{h}")
                  for h in range(NH)] for u in range(2)]
        zinv = [wpool.tile([P, NH], F32, tag=f"zi{u}", name=f"zi{u}")
                for u in range(2)]
        out_sb = [wpool.tile([P, D], DT, tag=f"out{u}", name=f"out{u}")
                  for u in range(2)]
        S_cp = wpool.tile([P, NH * NH * DVA], DT, tag="Scp", name="Scp")

        def proj_qk(s):
            """project q2 AND k for strip s in ONE packed M=96 group
            (head h block: rows 32h..+15 = q2_h, 32h+16..+31 = k_h), then
            redistribute rows into the padded qT/kT tiles.  The const
            feature rows (32h+16) are never touched after the init memset."""
            for k in range(NK):
                nc.tensor.matmul(
                    qk_ps, wqk_sb[:, k * GQ:(k + 1) * GQ],
                    hs_all[:, k * L + s * 512: k * L + (s + 1) * 512],
                    start=(k == 0), stop=(k == NK - 1))
            for h in range(NH):
                nc.scalar.activation(qT_sb[32 * h:32 * h + FDIM, s * 512:(s + 1) * 512],
                                     qk_ps[32 * h:32 * h + FDIM, :], _COPY)
            # k rows sit at 32h+16 (unaligned for DVE/ACT): stage the packed
            # block to SBUF, then redistribute via DMA (no alignment rules)
            nc.scalar.activation(pk_sb[:, s * 512:(s + 1) * 512], qk_ps, _COPY)
            for h in range(NH):
                nc.sync.dma_start(
                    kT_sb[32 * h:32 * h + FDIM, s * 512:(s + 1) * 512],
                    pk_sb[32 * h + FDIM:32 * h + 2 * FDIM, s * 512:(s + 1) * 512])

        def vproj(c):
            for k in range(NK):
                nc.tensor.matmul(
                    v_ps, hs_all[:, k * L + c * P: k * L + (c + 1) * P],
                    wv_sb[:, k * DV:(k + 1) * DV],
                    start=(k == 0), stop=(k == NK - 1))
            nc.scalar.activation(
                v_sb[:, c * NH * DVA:(c * NH + NH) * DVA].rearrange(
                    "p (h x) -> p h x", h=NH)[:, :, 0:HD],
                v_ps.rearrange("p (h x) -> p h x", h=NH), _COPY)

        def q2_trio(c, h):
            """PE replication of q2T rows for head h, then ONE DVE outer
            product over the adjacent (expA|expB) psum pair."""
            u = c % 2
            q2 = qT_sb[32 * h:32 * h + FDIM, c * P:(c + 1) * P]
            rep, expA, expB = q2_slots[h]
            nc.tensor.matmul(rep, rsel_sb[32 * h:32 * h + FDIM, 0:P], q2,
                             start=True, stop=True)
            nc.tensor.matmul(expA, rsel_sb[32 * h:32 * h + FDIM, P:2 * P], q2,
                             start=True, stop=True)
            nc.tensor.matmul(expB, rsel_sb[32 * h:32 * h + FDIM, 2 * P:3 * P], q2,
                             start=True, stop=True)
            rs = rep_sb[u][h]
            nc.vector.tensor_copy(rs[:], rep)
            nc.vector.scalar_tensor_tensor(
                fq2T[u][h][:].rearrange("p (a x) -> p a x", a=2),
                q2_pairs[h].rearrange("p (a x) -> p a x", a=2), 0.25,
                rs[:, None, :].broadcast_to([P, 2, P]),
                op0=_MULT, op1=_MULT)

        def fk2_mults(c):
            u = c % 2
            krm = k_rm[u][:].rearrange("p (h x) -> p h x", h=NH)[:, :, 0:FDIM]
            nc.gpsimd.tensor_mul(
                fk2[u][:].rearrange("p (h i j) -> p h i j", h=NH, i=FDIM),
                krm[:, :, :, None].broadcast_to([P, NH, FDIM, FDIM]),
                krm[:, :, None, :].broadcast_to([P, NH, FDIM, FDIM]))

        def sT_next(c):
            """sT + activation + mask for chunk c (consumed next iteration)."""
            u = c % 2
            for h in range(NH):
                nc.tensor.matmul(
                    sT_slots[h],
                    kT_sb[32 * h:32 * h + FDIM, c * P:(c + 1) * P],
                    qT_sb[32 * h:32 * h + FDIM, c * P:(c + 1) * P],
                    start=True, stop=True)
                nc.scalar.activation(attT[u][h][:], sT_slots[h], _SQUARE,
                                     bias=1.0, scale=0.5)
                nc.vector.scalar_tensor_tensor(attT[u][h][:], attT[u][h][:], 1.0,
                                               mask_sb[:], op0=_ADD, op1=_MULT)

        def upd(c):
            u = c % 2
            for h in range(NH):
                va = v_sb[:, (c * NH + h) * DVA:(c * NH + h + 1) * DVA]
                r0 = 32 * h
                nc.tensor.matmul(S_ps[h][r0:r0 + 17, 0:DVA],
                                 k_rm[u][:, r0:r0 + 17], va,
                                 start=True, stop=True)
                nc.tensor.matmul(S_ps[h][0:P, DVA:2 * DVA],
                                 fk2[u][:, h * 2 * P: h * 2 * P + P], va,
                                 start=True, stop=True)
                nc.tensor.matmul(S_ps[h][0:P, 2 * DVA:3 * DVA],
                                 fk2[u][:, h * 2 * P + P:(h + 1) * 2 * P], va,
                                 start=True, stop=True)

        def state_avz(c, h):
            # one CONTIGUOUS accumulation group per head: a start=True
            # re-arms the whole 2KB psum zero region, so groups sharing the
            # o_ps bank must never interleave
            u = c % 2
            sp = (c - 1) % 2
            og = o_ps[:, h * DVA:(h + 1) * DVA]
            r0 = 32 * h
            fa = fq2T[u][h][:, 0:P]
            fb = fq2T[u][h][:, P:2 * P]
            if c > 0:
                b0 = h * NH * DVA
                nc.tensor.matmul(og, qT_sb[r0:r0 + 17, c * P:(c + 1) * P],
                                 S_sb[sp][r0:r0 + 17, b0:b0 + DVA],
                                 start=True, stop=False)
                nc.tensor.matmul(og, fa, S_sb[sp][0:P, b0 + DVA:b0 + 2 * DVA],
                                 start=False, stop=False)
                nc.tensor.matmul(og, fb, S_sb[sp][0:P, b0 + 2 * DVA:b0 + 3 * DVA],
                                 start=False, stop=False)
            nc.tensor.matmul(og, attT[u][h][:],
                             v_sb[:, (c * NH + h) * DVA:(c * NH + h + 1) * DVA],
                             start=(c == 0), stop=True)

        def s_accum(c):
            with nc.allow_low_precision(reason="bf16 state accumulate, 2e-2 gate"):
                for h in range(NH):
                    nc.scalar.activation(
                        S_cp[:, h * NH * DVA:(h + 1) * NH * DVA], S_ps[h][:], _COPY)
                nc.gpsimd.tensor_add(S_sb[c % 2][:], S_cp[:],
                                     S_sb[(c - 1) % 2][:])

        def normalize(c):
            u = c % 2
            zc = o_ps.rearrange("p (h x) -> p h x", h=NH)[:, :, HD]
            nc.vector.reciprocal(zinv[u][:], zc)
            for h in range(NH):
                nc.scalar.activation(o_sb[u][h][:], o_ps[:, h * DVA: h * DVA + HD],
                                     _COPY, scale=zinv[u][:, h:h + 1])

        def o_transpose(c):
            u = c % 2
            for h in range(NH):
                nc.tensor.transpose(oT_slots[h], o_sb[u][h][:], identb[:])
                with nc.allow_low_precision(reason="oT copy bf16"):
                    nc.vector.tensor_copy(oT_sb[u][h][:], oT_slots[h])

        def oproj(c):
            u = c % 2
            for dc in range(3):
                ops = op_ps[(c * 3 + dc) % 2]
                for h in range(NH):
                    nc.tensor.matmul(ops, oT_sb[u][h][:],
                                     wo_sb[:, h * D + dc * 512: h * D + (dc + 1) * 512],
                                     start=(h == 0), stop=(h == NH - 1))
                nc.scalar.activation(out_sb[u][:, dc * 512:(dc + 1) * 512], ops, _COPY)
            nc.sync.dma_start(out[c * P:(c + 1) * P, :], out_sb[u][:])

        # PE warm-up: dummy matmuls ramp the clock p-state during the
        # initial DMA wait so real work starts at full speed
        warm = cpool.tile([P, P], DT, tag="warm")
        nc.vector.memset(warm[:], 0.0)
        for w in range(32):
            nc.tensor.matmul(slotA, identb[:], warm[:], start=True, stop=True)

        # ---- main loop (emission interleaved for PE continuity: every
        # cross-engine product is made one chunk ahead of its PE consumer)
        proj_qk(0)
        nc.sync.dma_start_transpose(k_rm[0][:], kT_sb[:, 0:P])
        q2_trio(0, 0)
        q2_trio(0, 1)
        q2_trio(0, 2)
        vproj(0)
        fk2_mults(0)
        sT_next(0)
        for c in range(NCH):
            n = c + 1
            if (c + 3) % 4 == 0 and c + 3 < NCH:
                proj_qk((c + 3) // 4)   # strip projected 2 iterations early
            upd(c)
            if n < NCH:
                nc.sync.dma_start_transpose(k_rm[n % 2][:],
                                            kT_sb[:, n * P:(n + 1) * P])
                q2_trio(n, 0)
            state_avz(c, 0)
            if n < NCH:
                q2_trio(n, 1)
            state_avz(c, 1)
            if n < NCH:
                q2_trio(n, 2)
            state_avz(c, 2)
            s_accum(c)
            normalize(c)
            if n < NCH:
                vproj(n)
                fk2_mults(n)
                sT_next(n)
            if c > 0:
                oproj(c - 1)
            o_transpose(c)
        oproj(NCH - 1)

    nc.compile()
    return nc


def _host_inputs(hidden_states, Wq, Wk, Wv, Wo):
    """Shard + lay out the full inputs into 8 per-core in_maps."""
    mask = (np.arange(P)[:, None] <= np.arange(P)[None, :]).astype(np.float32)

    # replication selectors, copies at row offsets 0/32/64 (one per head base)
    rsel = np.zeros((GQ, 3 * P + 2), dtype=np.float32)
    for h in range(3):
        rsel[32 * h + FDIM, 3 * P + 0] = 2.0   # qT const-row bias
        rsel[32 * h + FDIM, 3 * P + 1] = 1.0   # kT const-row bias
    for r0 in (0, 32, 64):
        for i in range(8):
            for j in range(FDIM):
                rsel[r0 + j, FDIM * i + j] = 1.0            # rep <- q2[j]
                rsel[r0 + i, P + FDIM * i + j] = 1.0        # expA <- q2[i]
                rsel[r0 + 8 + i, 2 * P + FDIM * i + j] = 1.0  # expB <- q2[i+8]

    in_maps = []
    for core in range(8):
        b, g = divmod(core, 4)
        heads = range(NH * g, NH * (g + 1))
        wqk_pack = np.zeros((D, GQ), dtype=np.float32)
        for i, h in enumerate(heads):
            wqk_pack[:, 32 * i: 32 * i + FDIM] = Wq[:, FDIM * h: FDIM * (h + 1)] * 0.5
            wqk_pack[:, 32 * i + FDIM: 32 * i + 2 * FDIM] = \
                Wk[:, FDIM * h: FDIM * (h + 1)]
        in_maps.append({
            "hsT": np.ascontiguousarray(hidden_states[b].T).astype(NPDT),
            "wqk": wqk_pack.astype(NPDT),
            "wv": np.ascontiguousarray(Wv[:, HD * NH * g: HD * NH * (g + 1)]).astype(NPDT),
            "wo": np.ascontiguousarray(Wo[HD * NH * g: HD * NH * (g + 1), :]).astype(NPDT),
            "maskd": mask.astype(NPDT),
            "rseld": rsel.astype(NPDT),
        })
    return in_maps


_NC = None


def _get_nc():
    global _NC
    if _NC is None:
        _NC = _build()
    return _NC


def run(hidden_states, Wq, Wk, Wv, Wo, trace=False, **trace_kwargs):
    nc = _get_nc()
    in_maps = _host_inputs(hidden_states, Wq, Wk, Wv, Wo)
    res = run_bass_kernel_spmd(nc, in_maps, core_ids=list(range(8)),
                               trace=trace, **trace_kwargs)
    out = np.zeros((B, L, D), dtype=np.float32)
    for core in range(8):
        out[core // 4] += res.results[core]["out"].astype(np.float32)
    return out, res


def kernel(hidden_states, Wq, Wk, Wv, Wo):
    out, _ = run(np.asarray(hidden_states, dtype=np.float32),
                 np.asarray(Wq, dtype=np.float32),
                 np.asarray(Wk, dtype=np.float32),
                 np.asarray(Wv, dtype=np.float32),
                 np.asarray(Wo, dtype=np.float32))
    return out


# revision 43
# speedup vs baseline: 1.0169x; 1.0169x over previous
"""Based linear-attention via chunked state form on 8 TRN2 NeuronCores.

Sharding: core c handles batch b = c // 4 and head-group g = c % 4
(3 of 12 heads).  Wq/Wk/Wv column-split by head, Wo row-split; each
core emits a partial [L, D] output and the host sums the 4 partials
per batch.

Algorithm: attn = 1 + s + 0.5 s^2 (s = q.k/sqrt(F)) is an exact
feature-map kernel phi(q).phi(k) with phi = [1, x, vec(x (x) x)]
(dim 1+16+256).  Chunked linear attention with C=128: per chunk the
intra part is one masked 128x128 quadratic block; the cross-chunk
part contracts phi(q) against a running state S = sum phi(k) (x)
[v | 1] per head ([2*17 + 256 rows, 129 cols]; column 128 carries
the causal normalizer z).  Everything is scaled 2x (attn2 = 2 + 2s
+ s^2 = (0.5*s2+1)^2 + 1 with s2 = 2s via Wq scaled by 0.5) so the
intra path stays one Square activation; the 2x cancels in o/z.

Matmul operands must sit at partition base 0/32/64 with equal bases
for lhsT/rhs, so qT/kT live in separate [96, L] tiles with head h in
rows 32h..32h+15; row 32h+16 holds the constant feature (2.0 in qT,
1.0 in kT), which merges the "1" feature into the 17-row ab state
block.  phi(q)'s 256 outer-product rows are built PE-side with 0/1
replication selectors + one DVE multiply (no psum->sbuf copy);
phi(k) gets l-major layout from one DMA transpose per chunk.
"""

import sys

sys.path.insert(0, "/opt/trn_rl_repo")

from contextlib import ExitStack

import ml_dtypes
import numpy as np

import concourse.bass as bass
import concourse.tile as tile
from concourse import bacc, mybir
from concourse.bass_utils import run_bass_kernel_spmd
from concourse.masks import make_identity

B, L, D = 2, 2048, 1536
H, FDIM, HD = 12, 16, 128
NH = 3            # heads per core
P = 128
NK = D // P       # 12 contraction tiles
NCH = L // P      # 16 chunks
GQ = 96           # padded q rows (3 heads x 32); same for k
DV = NH * HD      # 384 v cols per core
DVA = HD + 1      # 129: v columns + ones column per head

DT = mybir.dt.bfloat16
NPDT = ml_dtypes.bfloat16
F32 = mybir.dt.float32

_ADD = mybir.AluOpType.add
_MULT = mybir.AluOpType.mult
_SQUARE = mybir.ActivationFunctionType.Square
_COPY = mybir.ActivationFunctionType.Copy


def _build():
    nc = bacc.Bacc("TRN2", target_bir_lowering=False, debug=False, num_devices=8)

    hsT = nc.dram_tensor("hsT", [D, L], DT, kind="ExternalInput").ap()
    wqk = nc.dram_tensor("wqk", [D, GQ], DT, kind="ExternalInput").ap()
    wv = nc.dram_tensor("wv", [D, DV], DT, kind="ExternalInput").ap()
    wo = nc.dram_tensor("wo", [DV, D], DT, kind="ExternalInput").ap()
    maskd = nc.dram_tensor("maskd", [P, P], DT, kind="ExternalInput").ap()
    rseld = nc.dram_tensor("rseld", [GQ, 3 * P + 2], DT, kind="ExternalInput").ap()
    out = nc.dram_tensor("out", [L, D], DT, kind="ExternalOutput").ap()

    with tile.TileContext(nc, trace_sim=False) as tc, ExitStack() as ctx:
        cpool = ctx.enter_context(tc.tile_pool(name="consts", bufs=1))
        wqk_sb = cpool.tile([P, NK * GQ], DT, tag="wqk")
        wv_sb = cpool.tile([P, NK * DV], DT, tag="wv")
        wo_sb = cpool.tile([P, NH * D], DT, tag="wo")
        mask_sb = cpool.tile([P, P], DT, tag="mask")
        rsel_sb = cpool.tile([GQ, 3 * P + 2], DT, tag="rsel")
        ident = cpool.tile([P, P], F32, tag="ident")
        identb = cpool.tile([P, P], DT, tag="identb")
        hs_all = cpool.tile([P, NK * L], DT, tag="hs")     # [p, (k l)]
        qT_sb = cpool.tile([GQ, L], DT, tag="qT")
        pk_sb = cpool.tile([GQ, L], DT, tag="pk")
        kT_sb = cpool.tile([GQ, L], DT, tag="kT")
        v_sb = cpool.tile([P, NCH * NH * DVA], DT, tag="v")  # [l, (c h dv)]
        S_sb = [cpool.tile([P, NH * NH * DVA], DT, tag=f"S{u}", name=f"S{u}")
                for u in range(2)]   # head h at cols h*387..(h+1)*387

        # ---- input DMA (few big ops; strip 0 first so chunk 0 can start)
        nc.sync.dma_start(wqk_sb[:].rearrange("p (k g) -> p k g", k=NK),
                          wqk.rearrange("(k p) g -> p k g", p=P))
        SW = 4 * P
        hs3 = hs_all[:].rearrange("p (k l) -> p k l", k=NK)
        hsT3 = hsT.rearrange("(k p) l -> p k l", p=P)
        for s in range(4):
            nc.sync.dma_start(hs3[:, :, s * SW:(s + 1) * SW],
                              hsT3[:, :, s * SW:(s + 1) * SW])
            if s == 0:
                nc.sync.dma_start(rsel_sb[:], rseld)
                nc.sync.dma_start(mask_sb[:], maskd)
                nc.sync.dma_start(wv_sb[:].rearrange("p (k g) -> p k g", k=NK),
                                  wv.rearrange("(k p) g -> p k g", p=P))
            elif s == 1:
                nc.sync.dma_start(wo_sb[:].rearrange("p (h d) -> p h d", h=NH),
                                  wo.rearrange("(h p) d -> p h d", p=P))
        make_identity(nc, ident[:])
        make_identity(nc, identb[:])
        # constant feature rows: 2.0 in qT (the "2" of attn2), 1.0 in kT.
        # Whole 32-row groups (partition alignment); the real q/k rows are
        # overwritten by the projection copies before any read.
        for h in range(NH):
            nc.vector.memset(qT_sb[32 * h: 32 * h + 32, :], 2.0)
            nc.vector.memset(kT_sb[32 * h: 32 * h + 32, :], 1.0)
        # ones column of v_aug, all chunks/heads at once
        nc.vector.memset(
            v_sb[:].rearrange("p (c h x) -> p c h x", c=NCH, h=NH)[:, :, :, HD], 1.0)
        for u in range(2):
            nc.vector.memset(S_sb[u][:], 0.0)

        # ---- PSUM: exactly 8 banks
        ppool = ctx.enter_context(tc.tile_pool(name="ps", bufs=1, space="PSUM"))
        pb1 = ppool.tile([P, 512], F32, tag="pb1")   # qk + slots A,B,E1
        pb2 = ppool.tile([P, 512], F32, tag="pb2")   # v + slots C,C2,E2
        S_ps = [ppool.tile([P, NH * DVA], F32, tag=f"Sp{h}", name=f"Sp{h}")
                for h in range(NH)]                   # 3 banks
        o_ps = ppool.tile([P, NH * DVA], F32, tag="o_ps")  # 1 bank
        op_ps = [ppool.tile([P, 512], F32, tag=f"op{i}", name=f"op{i}")
                 for i in range(2)]                   # oproj double buffer

        qk_ps = op_ps[0][0:GQ, 0:512]   # time-shared with oproj buf 0
        slotA = pb1[:, 0:128]
        slotB = pb1[:, 128:256]
        slotC = pb1[:, 256:384]
        slotD = pb1[:, 384:512]
        v_ps = pb2[:, 0:DV]
        slotE = pb2[:, DV:DV + 128]
        sT_slots = [slotE, slotA, slotB]
        q2_slots = [[slotA, slotB, slotC], [slotE, slotC, slotD],
                    [slotE, slotA, slotB]]
        q2_pairs = [pb1[:, 128:384], pb1[:, 256:512], pb1[:, 0:256]]
        oT_slots = [slotC.bitcast(DT)[:, 0:P], slotD.bitcast(DT)[:, 0:P],
                    slotE.bitcast(DT)[:, 0:P]]

        # dead regions of the S psum ab-block must read as 0
        for h in range(NH):
            nc.vector.memset(S_ps[h][:, 0:DVA], 0.0)

        wpool = ctx.enter_context(tc.tile_pool(name="work", bufs=1))
        attT = [[wpool.tile([P, P], DT, tag=f"attT{u}{h}", name=f"attT{u}{h}")
                 for h in range(NH)] for u in range(2)]
        fq2T = [[wpool.tile([P, 2 * P], DT, tag=f"fq2T{u}{h}", name=f"fq2T{u}{h}")
                 for h in range(NH)] for u in range(2)]
        fk2 = [wpool.tile([P, NH * 2 * P], DT, tag=f"fk2{u}", name=f"fk2{u}")
               for u in range(2)]    # head h at cols h*256..(h+1)*256
        k_rm = [wpool.tile([P, GQ], DT, tag=f"krm{u}", name=f"krm{u}")
                for u in range(2)]
        rep_sb = [[wpool.tile([P, P], DT, tag=f"rep{u}{h}", name=f"rep{u}{h}")
                   for h in range(NH)] for u in range(2)]
        o_sb = [[wpool.tile([P, P], DT, tag=f"o{u}{h}", name=f"o{u}{h}")
                 for h in range(NH)] for u in range(2)]
        oT_sb = [[wpool.tile([P, P], DT, tag=f"oT{u}{h}", name=f"oT{u}{h}")
                  for h in range(NH)] for u in range(2)]
        zinv = [wpool.tile([P, NH], F32, tag=f"zi{u}", name=f"zi{u}")
                for u in range(2)]
        out_sb = [wpool.tile([P, D], DT, tag=f"out{u}", name=f"out{u}")
                  for u in range(2)]
        S_cp = wpool.tile([P, NH * NH * DVA], DT, tag="Scp", name="Scp")

        def proj_qk(s):
            """project q2 AND k for strip s in ONE packed M=96 group
            (head h block: rows 32h..+15 = q2_h, 32h+16..+31 = k_h), then
            redistribute rows into the padded qT/kT tiles.  The const
            feature rows (32h+16) are never touched after the init memset."""
            for k in range(NK):
                nc.tensor.matmul(
                    qk_ps, wqk_sb[:, k * GQ:(k + 1) * GQ],
                    hs_all[:, k * L + s * 512: k * L + (s + 1) * 512],
                    start=(k == 0), stop=(k == NK - 1))
            for h in range(NH):
                nc.scalar.activation(qT_sb[32 * h:32 * h + FDIM, s * 512:(s + 1) * 512],
                                     qk_ps[32 * h:32 * h + FDIM, :], _COPY)
            # k rows sit at 32h+16 (unaligned for DVE/ACT): stage the packed
            # block to SBUF, then redistribute via DMA (no alignment rules)
            nc.scalar.activation(pk_sb[:, s * 512:(s + 1) * 512], qk_ps, _COPY)
            for h in range(NH):
                nc.sync.dma_start(
                    kT_sb[32 * h:32 * h + FDIM, s * 512:(s + 1) * 512],
                    pk_sb[32 * h + FDIM:32 * h + 2 * FDIM, s * 512:(s + 1) * 512])

        def vproj(c):
            for k in range(NK):
                nc.tensor.matmul(
                    v_ps, hs_all[:, k * L + c * P: k * L + (c + 1) * P],
                    wv_sb[:, k * DV:(k + 1) * DV],
                    start=(k == 0), stop=(k == NK - 1))
            nc.scalar.activation(
                v_sb[:, c * NH * DVA:(c * NH + NH) * DVA].rearrange(
                    "p (h x) -> p h x", h=NH)[:, :, 0:HD],
                v_ps.rearrange("p (h x) -> p h x", h=NH), _COPY)

        def q2_trio(c, h):
            """PE replication of q2T rows for head h, then ONE DVE outer
            product over the adjacent (expA|expB) psum pair."""
            u = c % 2
            q2 = qT_sb[32 * h:32 * h + FDIM, c * P:(c + 1) * P]
            rep, expA, expB = q2_slots[h]
            nc.tensor.matmul(rep, rsel_sb[32 * h:32 * h + FDIM, 0:P], q2,
                             start=True, stop=True)
            nc.tensor.matmul(expA, rsel_sb[32 * h:32 * h + FDIM, P:2 * P], q2,
                             start=True, stop=True)
            nc.tensor.matmul(expB, rsel_sb[32 * h:32 * h + FDIM, 2 * P:3 * P], q2,
                             start=True, stop=True)
            rs = rep_sb[u][h]
            nc.vector.tensor_copy(rs[:], rep)
            nc.vector.scalar_tensor_tensor(
                fq2T[u][h][:].rearrange("p (a x) -> p a x", a=2),
                q2_pairs[h].rearrange("p (a x) -> p a x", a=2), 0.25,
                rs[:, None, :].broadcast_to([P, 2, P]),
                op0=_MULT, op1=_MULT)

        def fk2_mults(c):
            u = c % 2
            krm = k_rm[u][:].rearrange("p (h x) -> p h x", h=NH)[:, :, 0:FDIM]
            nc.gpsimd.tensor_mul(
                fk2[u][:].rearrange("p (h i j) -> p h i j", h=NH, i=FDIM),
                krm[:, :, :, None].broadcast_to([P, NH, FDIM, FDIM]),
                krm[:, :, None, :].broadcast_to([P, NH, FDIM, FDIM]))

        def sT_next(c):
            """sT + activation + mask for chunk c (consumed next iteration)."""
            u = c % 2
            for h in range(NH):
                nc.tensor.matmul(
                    sT_slots[h],
                    kT_sb[32 * h:32 * h + FDIM, c * P:(c + 1) * P],
                    qT_sb[32 * h:32 * h + FDIM, c * P:(c + 1) * P],
                    start=True, stop=True)
                nc.scalar.activation(attT[u][h][:], sT_slots[h], _SQUARE,
                                     bias=1.0, scale=0.5)
                nc.vector.scalar_tensor_tensor(attT[u][h][:], attT[u][h][:], 1.0,
                                               mask_sb[:], op0=_ADD, op1=_MULT)

        def upd(c):
            u = c % 2
            for h in range(NH):
                va = v_sb[:, (c * NH + h) * DVA:(c * NH + h + 1) * DVA]
                r0 = 32 * h
                nc.tensor.matmul(S_ps[h][r0:r0 + 17, 0:DVA],
                                 k_rm[u][:, r0:r0 + 17], va,
                                 start=True, stop=True)
                nc.tensor.matmul(S_ps[h][0:P, DVA:2 * DVA],
                                 fk2[u][:, h * 2 * P: h * 2 * P + P], va,
                                 start=True, stop=True)
                nc.tensor.matmul(S_ps[h][0:P, 2 * DVA:3 * DVA],
                                 fk2[u][:, h * 2 * P + P:(h + 1) * 2 * P], va,
                                 start=True, stop=True)

        def state_avz(c, h):
            # one CONTIGUOUS accumulation group per head: a start=True
            # re-arms the whole 2KB psum zero region, so groups sharing the
            # o_ps bank must never interleave
            u = c % 2
            sp = (c - 1) % 2
            og = o_ps[:, h * DVA:(h + 1) * DVA]
            r0 = 32 * h
            fa = fq2T[u][h][:, 0:P]
            fb = fq2T[u][h][:, P:2 * P]
            if c > 0:
                b0 = h * NH * DVA
                nc.tensor.matmul(og, qT_sb[r0:r0 + 17, c * P:(c + 1) * P],
                                 S_sb[sp][r0:r0 + 17, b0:b0 + DVA],
                                 start=True, stop=False)
                nc.tensor.matmul(og, fa, S_sb[sp][0:P, b0 + DVA:b0 + 2 * DVA],
                                 start=False, stop=False)
                nc.tensor.matmul(og, fb, S_sb[sp][0:P, b0 + 2 * DVA:b0 + 3 * DVA],
                                 start=False, stop=False)
            nc.tensor.matmul(og, attT[u][h][:],
                             v_sb[:, (c * NH + h) * DVA:(c * NH + h + 1) * DVA],
                             start=(c == 0), stop=True)

        def s_accum(c):
            with nc.allow_low_precision(reason="bf16 state accumulate, 2e-2 gate"):
                for h in range(NH):
                    nc.scalar.activation(
                        S_cp[:, h * NH * DVA:(h + 1) * NH * DVA], S_ps[h][:], _COPY)
                nc.gpsimd.tensor_add(S_sb[c % 2][:], S_cp[:],
                                     S_sb[(c - 1) % 2][:])

        def normalize(c):
            u = c % 2
            zc = o_ps.rearrange("p (h x) -> p h x", h=NH)[:, :, HD]
            nc.vector.reciprocal(zinv[u][:], zc)
            for h in range(NH):
                nc.scalar.activation(o_sb[u][h][:], o_ps[:, h * DVA: h * DVA + HD],
                                     _COPY, scale=zinv[u][:, h:h + 1])

        def o_transpose(c):
            u = c % 2
            for h in range(NH):
                nc.tensor.transpose(oT_slots[h], o_sb[u][h][:], identb[:])
                with nc.allow_low_precision(reason="oT copy bf16"):
                    nc.vector.tensor_copy(oT_sb[u][h][:], oT_slots[h])

        def oproj(c):
            u = c % 2
            for dc in range(3):
                ops = op_ps[(c * 3 + dc) % 2]
                for h in range(NH):
                    nc.tensor.matmul(ops, oT_sb[u][h][:],
                                     wo_sb[:, h * D + dc * 512: h * D + (dc + 1) * 512],
                                     start=(h == 0), stop=(h == NH - 1))
                with nc.allow_low_precision(reason="out copy bf16"):
                    nc.vector.tensor_copy(out_sb[u][:, dc * 512:(dc + 1) * 512], ops)
            nc.sync.dma_start(out[c * P:(c + 1) * P, :], out_sb[u][:])

        # PE warm-up: dummy matmuls ramp the clock p-state during the
        # initial DMA wait so real work starts at full speed
        warm = cpool.tile([P, P], DT, tag="warm")
        nc.vector.memset(warm[:], 0.0)
        for w in range(32):
            nc.tensor.matmul(slotA, identb[:], warm[:], start=True, stop=True)

        # ---- main loop (emission interleaved for PE continuity: every
        # cross-engine product is made one chunk ahead of its PE consumer)
        proj_qk(0)
        nc.sync.dma_start_transpose(k_rm[0][:], kT_sb[:, 0:P])
        q2_trio(0, 0)
        q2_trio(0, 1)
        q2_trio(0, 2)
        vproj(0)
        fk2_mults(0)
        sT_next(0)
        for c in range(NCH):
            n = c + 1
            if (c + 3) % 4 == 0 and c + 3 < NCH:
                proj_qk((c + 3) // 4)   # strip projected 2 iterations early
            upd(c)
            if n < NCH:
                nc.sync.dma_start_transpose(k_rm[n % 2][:],
                                            kT_sb[:, n * P:(n + 1) * P])
                q2_trio(n, 0)
            state_avz(c, 0)
            if n < NCH:
                q2_trio(n, 1)
            state_avz(c, 1)
            if n < NCH:
                q2_trio(n, 2)
            state_avz(c, 2)
            s_accum(c)
            normalize(c)
            if n < NCH:
                vproj(n)
                fk2_mults(n)
                sT_next(n)
            if c > 0:
                oproj(c - 1)
            o_transpose(c)
        oproj(NCH - 1)

    nc.compile()
    return nc


def _host_inputs(hidden_states, Wq, Wk, Wv, Wo):
    """Shard + lay out the full inputs into 8 per-core in_maps."""
    mask = (np.arange(P)[:, None] <= np.arange(P)[None, :]).astype(np.float32)

    # replication selectors, copies at row offsets 0/32/64 (one per head base)
    rsel = np.zeros((GQ, 3 * P + 2), dtype=np.float32)
    for h in range(3):
        rsel[32 * h + FDIM, 3 * P + 0] = 2.0   # qT const-row bias
        rsel[32 * h + FDIM, 3 * P + 1] = 1.0   # kT const-row bias
    for r0 in (0, 32, 64):
        for i in range(8):
            for j in range(FDIM):
                rsel[r0 + j, FDIM * i + j] = 1.0            # rep <- q2[j]
                rsel[r0 + i, P + FDIM * i + j] = 1.0        # expA <- q2[i]
                rsel[r0 + 8 + i, 2 * P + FDIM * i + j] = 1.0  # expB <- q2[i+8]

    in_maps = []
    for core in range(8):
        b, g = divmod(core, 4)
        heads = range(NH * g, NH * (g + 1))
        wqk_pack = np.zeros((D, GQ), dtype=np.float32)
        for i, h in enumerate(heads):
            wqk_pack[:, 32 * i: 32 * i + FDIM] = Wq[:, FDIM * h: FDIM * (h + 1)] * 0.5
            wqk_pack[:, 32 * i + FDIM: 32 * i + 2 * FDIM] = \
                Wk[:, FDIM * h: FDIM * (h + 1)]
        in_maps.append({
            "hsT": np.ascontiguousarray(hidden_states[b].T).astype(NPDT),
            "wqk": wqk_pack.astype(NPDT),
            "wv": np.ascontiguousarray(Wv[:, HD * NH * g: HD * NH * (g + 1)]).astype(NPDT),
            "wo": np.ascontiguousarray(Wo[HD * NH * g: HD * NH * (g + 1), :]).astype(NPDT),
            "maskd": mask.astype(NPDT),
            "rseld": rsel.astype(NPDT),
        })
    return in_maps


_NC = None


def _get_nc():
    global _NC
    if _NC is None:
        _NC = _build()
    return _NC


def run(hidden_states, Wq, Wk, Wv, Wo, trace=False, **trace_kwargs):
    nc = _get_nc()
    in_maps = _host_inputs(hidden_states, Wq, Wk, Wv, Wo)
    res = run_bass_kernel_spmd(nc, in_maps, core_ids=list(range(8)),
                               trace=trace, **trace_kwargs)
    out = np.zeros((B, L, D), dtype=np.float32)
    for core in range(8):
        out[core // 4] += res.results[core]["out"].astype(np.float32)
    return out, res


def kernel(hidden_states, Wq, Wk, Wv, Wo):
    out, _ = run(np.asarray(hidden_states, dtype=np.float32),
                 np.asarray(Wq, dtype=np.float32),
                 np.asarray(Wk, dtype=np.float32),
                 np.asarray(Wv, dtype=np.float32),
                 np.asarray(Wo, dtype=np.float32))
    return out
